# revision 15
# baseline (speedup 1.0000x reference)
"""Differential cross-attention Trainium2 kernel (8 NeuronCores).

Sharding: 8 cores = (batch b = c//2) x (query half = c%2). Each core
computes all 8 heads for its 512 queries against all 1024 keys of its
batch.

v3: all matmuls bf16 (fp32 PE matmul costs 4 cycles/column vs 1).
Heads pair-interleaved on host (head hp at partitions 0:64, head hp+4
at 64:128 of tile hp) so the two score matmuls of a differential pair
occupy disjoint PE row groups (row tiling -> concurrent on HW) and
exp / bias-multiply batch over [128, 1024]. V projection writes
pair-blocks [v_h2 | v_h1] straight into ve2 via one strided
tensor_tensor per k-tile (wv/bv column order is swapped per pair on
host); ve2 = [v2 | v1 | 1] serves both PV matmuls. Emission is
software-pipelined: Q/K projections per pair, V interleaved into pair
0's k-loop, PV skewed one k-tile behind exp/multiply, transposes of
pair hp-1 folded between pair hp's projections. Engine split: ACT does
exp + x2 scaling + xcat_T evacuation, DVE does projection evacuation /
combines / most bias-multiplies, Pool (GPSIMD) takes 2 of 8
bias-multiplies per pair. Differential combine per-q scalars:
  x1 = (1+alpha)/S1 * U1V1 - alpha*lam/S2 * U2V1,  x2 = U2V2/S2.
"""
import sys
sys.path.insert(0, "/opt/trn_rl_repo")
import numpy as np

DIM = 512
H = 8
HD = 64
NQC = 512
NKV = 1024
MAX_DIST = 128
LAMBDA_INIT = 0.8
N_CORES = 8
SCALE = HD ** -0.5
POOL_MULT_MS = (1, 4, 7)   # k-tiles whose bias-multiply runs on GPSIMD

_COMPILED = {}


def _pv_matmuls(nc, pvs, u, ve2, hp, m, first, last):
    for qt in range(4):
        q0 = 128 * qt
        pv = pvs[qt]
        st = first and (qt % 2 == 0)
        sp = last and (qt % 2 == 1)
        nc.tensor.matmul(pv[:, 0:65], lhsT=u[:, 0, q0:q0 + 128],
                         rhs=ve2[:, hp, m, 64:129], start=st, stop=False)
        nc.tensor.matmul(pv[:, 65:194], lhsT=u[:, 1, q0:q0 + 128],
                         rhs=ve2[:, hp, m, :], start=False, stop=sp)


def _build_body(nc, tc, mybir, make_identity, tensors):
    f32 = mybir.dt.float32
    bf16 = mybir.dt.bfloat16
    AF = mybir.ActivationFunctionType
    OP = mybir.AluOpType

    xq_T = tensors["xq_T"]
    xkv_T = tensors["xkv_T"]
    wq, wk, wv, wp = (tensors[k] for k in ("wq", "wk", "wv", "wp"))
    bq, bk, bv, bp = (tensors[k] for k in ("bq", "bk", "bv", "bp"))
    alpha = tensors["alpha"]
    lam_in = tensors["lam"]
    biasT = tensors["biasT"]
    out_T = tensors["out_T"]

    with (
        tc.tile_pool(name="const", bufs=1) as cpool,
        tc.tile_pool(name="work", bufs=1) as wpool,
        tc.tile_pool(name="stream", bufs=6) as spool,
        tc.tile_pool(name="bias", bufs=10) as bpool,
    ):
        # One merged DMA for the small constants, then inputs in
        # critical-path order: xq+wq feed the first projection, the kh=0
        # half of xkv + wk feed the first score matmuls, wv feeds V inside
        # pair 0's loop; wp is not needed until the very end.
        sm_t = cpool.tile([128, 20], f32, tag="smalls")
        nc.sync.dma_start(out=sm_t[:], in_=tensors["smalls"][:])
        bq_t = sm_t[:, 0:4]
        bk_t = sm_t[:, 4:8]
        bp_t = sm_t[:, 8:12]
        al_t = sm_t[:, 12:16]
        lam_t = sm_t[:, 16:20]
        bv_t = cpool.tile([128, DIM], f32, tag="bv")

        wq_t = cpool.tile([128, 4, DIM], bf16, tag="wq")
        wk_t = cpool.tile([128, 4, DIM], bf16, tag="wk")
        wv_t = cpool.tile([128, 4, DIM], bf16, tag="wv")
        wp_t = cpool.tile([128, 4, DIM], bf16, tag="wp")
        xq_t = wpool.tile([128, 4, NQC], bf16, tag="xq")
        xkv_t = wpool.tile([128, 4, NKV], bf16, tag="xkv")
        xq_r = xq_T[:].rearrange("(c p) n -> p c n", p=128)
        xkv_r = xkv_T[:].rearrange("(c p) n -> p c n", p=128)
        wq_r = wq[:].rearrange("(c p) o -> p c o", p=128)
        wk_r = wk[:].rearrange("(c p) o -> p c o", p=128)
        for c in range(4):
            nc.sync.dma_start(out=xq_t[:, c, :], in_=xq_r[:, c, :])
            nc.sync.dma_start(out=wq_t[:, c, :], in_=wq_r[:, c, :])
        for c in range(4):
            nc.sync.dma_start(out=xkv_t[:, c, 0:512], in_=xkv_r[:, c, 0:512])
            nc.sync.dma_start(out=wk_t[:, c, :], in_=wk_r[:, c, :])
        for c in range(4):
            nc.sync.dma_start(out=xkv_t[:, c, 512:1024],
                              in_=xkv_r[:, c, 512:1024])
        for c in range(4):
            nc.sync.dma_start(
                out=wv_t[:, c, :],
                in_=wv[:].rearrange("(c p) o -> p c o", p=128)[:, c, :])
        nc.sync.dma_start(out=bv_t[:], in_=bv[:])

        al1_t = cpool.tile([128, 4], f32, tag="al1")
        nc.vector.tensor_scalar(out=al1_t[:], in0=al_t[:], scalar1=1.0,
                                scalar2=None, op0=OP.add)
        alam_t = cpool.tile([128, 4, 4], f32, tag="alam")
        for hp in range(4):
            nc.vector.tensor_scalar(out=alam_t[:, hp, :], in0=al_t[:],
                                    scalar1=lam_t[:, hp:hp + 1], scalar2=None,
                                    op0=OP.mult)
        ident = cpool.tile([128, 128], bf16, tag="ident")
        make_identity(nc, ident[:])
        # Preload the exp spline tables during the initial DMA window.
        warm = cpool.tile([128, 2], f32, tag="warm")
        nc.vector.memset(warm[:, 0:1], 0.0)
        nc.scalar.activation(warm[:, 1:2], warm[:, 0:1], AF.Exp)

        ve2 = cpool.tile([128, 4, 8, 129], bf16, tag="ve2")
        q_sb = cpool.tile([128, 4, NQC], bf16, tag="qsb")
        k_sb = cpool.tile([128, 4, NKV], bf16, tag="ksb")
        xcat = wpool.tile([128, 4, DIM], bf16, tag="xcat")
        xcat_T = wpool.tile([128, 4, NQC], bf16, tag="xcatT")
        nc.vector.memset(ve2[:, :, :, 128:129], 1.0)
        bv_r = bv_t[:].rearrange("p (a b) -> p a b", a=4)

        with (
            tc.tile_pool(name="psum", bufs=2, space="PSUM") as ppool,
            tc.tile_pool(name="psc", bufs=2, space="PSUM") as scpool,
            tc.tile_pool(name="psacc", bufs=2, space="PSUM") as papool,
        ):
            for hp in range(4):
                # Q/K projections for this head pair (dims pair-interleaved
                # on host: head hp at rows 0:64, head hp+4 at rows 64:128).
                ps = ppool.tile([128, NQC], f32, tag="proj")
                for c in range(4):
                    nc.tensor.matmul(ps[:], lhsT=wq_t[:, c, 128 * hp:128 * (hp + 1)],
                                     rhs=xq_t[:, c, :], start=(c == 0), stop=(c == 3))
                nc.vector.tensor_scalar(out=q_sb[:, hp, :], in0=ps[:],
                                        scalar1=bq_t[:, hp:hp + 1], scalar2=None,
                                        op0=OP.add)
                for kh in range(2):
                    ps = ppool.tile([128, NQC], f32, tag="proj")
                    for c in range(4):
                        nc.tensor.matmul(
                            ps[:], lhsT=wk_t[:, c, 128 * hp:128 * (hp + 1)],
                            rhs=xkv_t[:, c, 512 * kh:512 * (kh + 1)],
                            start=(c == 0), stop=(c == 3))
                    nc.vector.tensor_scalar(out=k_sb[:, hp, 512 * kh:512 * (kh + 1)],
                                            in0=ps[:], scalar1=bk_t[:, hp:hp + 1],
                                            scalar2=None, op0=OP.add)
                if hp > 0:
                    _transpose_pair(nc, ppool, xcat, xcat_T, ident, hp - 1,
                                    f32, bf16, AF)

                pvbanks = []
                for _qb in range(2):
                    pv_bank = papool.tile([128, 388], f32, tag="pvacc")
                    pvbanks.append(pv_bank)
                pvs = [pvbanks[qt // 2][:, 194 * (qt % 2):194 * (qt % 2 + 1)]
                       for qt in range(4)]
                prev = None
                for m in range(8):
                    btp = bpool.tile([128, 2, NQC], bf16, tag="biasin")
                    nc.sync.dma_start(
                        out=btp[:],
                        in_=biasT[hp, m, :, :, :].rearrange("t p n -> p t n"))
                    # Row-packed score matmuls: the two heads of the pair use
                    # disjoint 64-row PE strips and adjacent PSUM banks.
                    ss = scpool.tile([128, 2, NQC], f32, tag="scores")
                    nc.tensor.matmul(
                        ss[:, 0, :],
                        lhsT=k_sb[0:64, hp, 128 * m:128 * (m + 1)],
                        rhs=q_sb[0:64, hp, :], start=True, stop=True)
                    nc.tensor.matmul(
                        ss[:, 1, :],
                        lhsT=k_sb[64:128, hp, 128 * m:128 * (m + 1)],
                        rhs=q_sb[64:128, hp, :], start=True, stop=True)
                    if hp == 0:
                        # V projection for k-tile m, written straight into
                        # ve2 pair blocks ([v_h2 | v_h1] column order from
                        # the host-side wv permutation).
                        vps = ppool.tile([128, DIM], f32, tag="proj")
                        for c in range(4):
                            nc.tensor.matmul(
                                vps[:], lhsT=xkv_t[:, c, 128 * m:128 * (m + 1)],
                                rhs=wv_t[:, c, :], start=(c == 0), stop=(c == 3))
                        nc.vector.tensor_tensor(
                            out=ve2[:, :, m, 0:128],
                            in0=vps[:].rearrange("p (a b) -> p a b", a=4),
                            in1=bv_r, op=OP.add)
                    uq = spool.tile([128, 2, NQC], bf16, tag="uq")
                    nc.scalar.activation(uq[:], ss[:], AF.Exp)
                    u = spool.tile([128, 2, NQC], bf16, tag="u")
                    eng = nc.gpsimd if m in POOL_MULT_MS else nc.vector
                    eng.tensor_tensor(out=u[:], in0=uq[:], in1=btp[:], op=OP.mult)
                    if prev is not None:
                        _pv_matmuls(nc, pvs, prev[0], ve2, hp, prev[1],
                                    first=(prev[1] == 0), last=False)
                    prev = (u, m)
                _pv_matmuls(nc, pvs, prev[0], ve2, hp, prev[1],
                            first=False, last=True)

                for qt in range(4):
                    pv = pvs[qt]
                    rs1 = spool.tile([128, 1], f32, tag="rs1")
                    rs2 = spool.tile([128, 1], f32, tag="rs2")
                    nc.vector.reciprocal(rs1[:], pv[:, 64:65])
                    nc.vector.reciprocal(rs2[:], pv[:, 193:194])
                    tmp1 = spool.tile([128, 64], f32, tag="tmp1")
                    nc.vector.tensor_scalar(out=tmp1[:], in0=pv[:, 0:64],
                                            scalar1=rs1[:],
                                            scalar2=al1_t[:, qt:qt + 1],
                                            op0=OP.mult, op1=OP.mult)
                    tmp2 = spool.tile([128, 64], f32, tag="tmp2")
                    nc.vector.tensor_scalar(out=tmp2[:], in0=pv[:, 129:193],
                                            scalar1=rs2[:],
                                            scalar2=alam_t[:, hp, qt:qt + 1],
                                            op0=OP.mult, op1=OP.mult)
                    nc.vector.tensor_tensor(out=xcat[:, qt, 128 * hp:128 * hp + 64],
                                            in0=tmp1[:], in1=tmp2[:],
                                            op=OP.subtract)
                    nc.scalar.activation(xcat[:, qt, 128 * hp + 64:128 * (hp + 1)],
                                         pv[:, 65:129], AF.Copy, scale=rs2[:])
            _transpose_pair(nc, ppool, xcat, xcat_T, ident, 3, f32, bf16, AF)

        for c in range(4):
            nc.sync.dma_start(
                out=wp_t[:, c, :],
                in_=wp[:].rearrange("(c p) o -> p c o", p=128)[:, c, :])
        with tc.tile_pool(name="psout", bufs=2, space="PSUM") as opool:
            for t in range(4):
                ps = opool.tile([128, NQC], f32, tag="oproj")
                for c in range(4):
                    nc.tensor.matmul(ps[:], lhsT=wp_t[:, c, 128 * t:128 * (t + 1)],
                                     rhs=xcat_T[:, c, :], start=(c == 0), stop=(c == 3))
                ot = spool.tile([128, NQC], bf16, tag="otile")
                nc.vector.tensor_scalar(out=ot[:], in0=ps[:],
                                        scalar1=bp_t[:, t:t + 1], scalar2=None,
                                        op0=OP.add)
                nc.sync.dma_start(
                    out=out_T[:].rearrange("(c p) n -> p c n", p=128)[:, t, :],
                    in_=ot[:])


def _transpose_pair(nc, ppool, xcat, xcat_T, ident, dit, f32, bf16, AF):
    """Transpose xcat[:, :, 128*dit:128*(dit+1)] into xcat_T[:, dit, :] via
    four PE transposes packed into one PSUM bank (bf16 view of a proj tile),
    then a single ACT copy."""
    pp = ppool.tile([128, NQC], f32, tag="proj")
    ppv = pp[:].bitcast(bf16)
    for qt in range(4):
        nc.tensor.transpose(out=ppv[:, 128 * qt:128 * (qt + 1)],
                            in_=xcat[:, qt, 128 * dit:128 * (dit + 1)],
                            identity=ident[:])
    nc.scalar.activation(xcat_T[:, dit, :], ppv[:, 0:NQC], AF.Copy)


def _build(reps=1):
    import concourse.bacc as bacc
    import concourse.mybir as mybir
    from concourse.tile import TileContext
    from concourse.masks import make_identity

    f32 = mybir.dt.float32
    bf16 = mybir.dt.bfloat16
    nc = bacc.Bacc("TRN2", target_bir_lowering=False, debug=False,
                   num_devices=N_CORES)
    tensors = {}
    for name, shape, dt in (
        ("xq_T", [DIM, NQC], bf16),
        ("xkv_T", [DIM, NKV], bf16),
        ("wq", [DIM, DIM], bf16),
        ("wk", [DIM, DIM], bf16),
        ("wv", [DIM, DIM], bf16),
        ("wp", [DIM, DIM], bf16),
        ("bq", [128, 4], f32),
        ("bk", [128, 4], f32),
        ("bv", [128, DIM], f32),
        ("bp", [128, 4], f32),
        ("alpha", [128, 4], f32),
        ("lam", [128, 4], f32),
        ("biasT", [4, 8, 2, 128, NQC], bf16),
    ):
        tensors[name] = nc.dram_tensor(name, shape, dt, kind="ExternalInput")
    tensors["out_T"] = nc.dram_tensor("out_T", [DIM, NQC], bf16,
                                      kind="ExternalOutput")

    with TileContext(nc) as tc:
        for _rep in range(reps):
            _build_body(nc, tc, mybir, make_identity, tensors)
    nc.compile()
    return nc


def _get_kernel(reps=1):
    key = f"k{reps}"
    if key not in _COMPILED:
        _COMPILED[key] = _build(reps)
    return _COMPILED[key]


def _to_bf16(a):
    import jax.numpy as jnp
    return np.asarray(jnp.asarray(np.asarray(a, dtype=np.float32),
                                  dtype=jnp.bfloat16))


# Head pair-interleave: projection output dim order becomes, per pair hp,
# [64 dims of head hp, 64 dims of head hp+4] for Q/K (and the xcat/Wp input
# side), and [head hp+4, head hp] for V (ve2 block order).
_PERM = np.concatenate(
    [np.r_[64 * hp:64 * (hp + 1), 64 * (hp + 4):64 * (hp + 5)] for hp in range(4)])
_PERM_V = np.concatenate(
    [np.r_[64 * (hp + 4):64 * (hp + 5), 64 * hp:64 * (hp + 1)] for hp in range(4)])


def _prep_inputs(x_q, x_kv, coords_q, coords_k, alpha_map,
                 Wq, bq, Wk, bk, Wv, bv,
                 lambda_q1, lambda_k1, lambda_q2, lambda_k2,
                 rpe_table, Wp, bp):
    x_q = np.asarray(x_q, dtype=np.float32)
    x_kv = np.asarray(x_kv, dtype=np.float32)
    coords_q = np.asarray(coords_q)
    coords_k = np.asarray(coords_k)
    alpha_map = np.asarray(alpha_map, dtype=np.float32)
    rpe = np.asarray(rpe_table, dtype=np.float32)

    lam1 = np.exp(np.sum(np.asarray(lambda_q1) * np.asarray(lambda_k1), axis=-1))
    lam2 = np.exp(np.sum(np.asarray(lambda_q2) * np.asarray(lambda_k2), axis=-1))
    lam = (lam1 - lam2 + LAMBDA_INIT).astype(np.float32)
    lam_rep = np.ascontiguousarray(np.tile(lam[None, :], (128, 1)))

    p = _PERM
    wq_l = _to_bf16((np.asarray(Wq, dtype=np.float32).T * SCALE)[:, p])
    wk_l = _to_bf16(np.asarray(Wk, dtype=np.float32).T[:, p])
    wv_l = _to_bf16(np.asarray(Wv, dtype=np.float32).T[:, _PERM_V])
    wp_l = _to_bf16(np.asarray(Wp, dtype=np.float32).T[p, :])
    bq_l = np.ascontiguousarray(
        (np.asarray(bq, dtype=np.float32) * SCALE)[p].reshape(4, 128).T)
    bk_l = np.ascontiguousarray(np.asarray(bk, dtype=np.float32)[p].reshape(4, 128).T)
    bv_l = np.ascontiguousarray(
        np.tile(np.asarray(bv, dtype=np.float32)[None, _PERM_V], (128, 1)))
    bp_l = np.ascontiguousarray(np.asarray(bp, dtype=np.float32).reshape(4, 128).T)

    in_maps = []
    for c in range(N_CORES):
        b, qh = divmod(c, 2)
        qsl = slice(qh * NQC, (qh + 1) * NQC)
        cq = coords_q[b, qsl]
        ck = coords_k[b]
        rel = cq[:, None, :] - ck[None, :, :] + MAX_DIST
        rel = np.clip(rel, 0, 2 * MAX_DIST)
        idx = rel[..., 0] * (2 * MAX_DIST + 1) + rel[..., 1]
        bias = np.exp(rpe[idx])                                # [512q, 1024k, 8]
        biasT = bias.transpose(2, 1, 0)                        # [8, 1024k, 512q]
        biasT = np.ascontiguousarray(
            biasT.reshape(2, 4, 8, 128, NQC).transpose(1, 2, 0, 3, 4))  # [hp, m, 2, 128, q]
        in_maps.append({
            "xq_T": _to_bf16(x_q[b, qsl].T),
            "xkv_T": _to_bf16(x_kv[b].T),
            "wq": wq_l, "wk": wk_l, "wv": wv_l, "wp": wp_l,
            "bq": bq_l, "bk": bk_l, "bv": bv_l, "bp": bp_l,
            "alpha": np.ascontiguousarray(alpha_map[b, qsl, 0].reshape(4, 128).T),
            "lam": lam_rep,
            "biasT": _to_bf16(biasT),
        })
    return in_maps


def kernel(x_q, x_kv, coords_q, coords_k, alpha_map,
           Wq, bq, Wk, bk, Wv, bv,
           lambda_q1, lambda_k1, lambda_q2, lambda_k2,
           rpe_table, Wp, bp):
    from concourse.bass_utils import run_bass_kernel_spmd

    nc = _get_kernel()
    in_maps = _prep_inputs(x_q, x_kv, coords_q, coords_k, alpha_map,
                           Wq, bq, Wk, bk, Wv, bv,
                           lambda_q1, lambda_k1, lambda_q2, lambda_k2,
                           rpe_table, Wp, bp)
    res = run_bass_kernel_spmd(nc, in_maps, list(range(N_CORES)))
    B = np.asarray(x_q).shape[0]
    out = np.zeros((B, 2 * NQC, DIM), dtype=np.float32)
    for c in range(N_CORES):
        b, qh = divmod(c, 2)
        out[b, qh * NQC:(qh + 1) * NQC] = (
            res.results[c]["out_T"].astype(np.float32).T)
    return out


# revision 22
# speedup vs baseline: 23.2285x; 23.2285x over previous
"""Differential cross-attention Trainium2 kernel (8 NeuronCores).

Sharding: 8 cores = (batch b = c//2) x (query half = c%2). Each core
computes all 8 heads for its 512 queries against all 1024 keys of its
batch.

v3: all matmuls bf16 (fp32 PE matmul costs 4 cycles/column vs 1).
Heads pair-interleaved on host (head hp at partitions 0:64, head hp+4
at 64:128 of tile hp) so the two score matmuls of a differential pair
occupy disjoint PE row groups (row tiling -> concurrent on HW) and
exp / bias-multiply batch over [128, 1024]. V projection writes
pair-blocks [v_h2 | v_h1] straight into ve2 via one strided
tensor_tensor per k-tile (wv/bv column order is swapped per pair on
host); ve2 = [v2 | v1 | 1] serves both PV matmuls. Emission is
software-pipelined: Q/K projections per pair, V interleaved into pair
0's k-loop, PV skewed one k-tile behind exp/multiply, transposes of
pair hp-1 folded between pair hp's projections. Engine split: ACT does
exp + x2 scaling + xcat_T evacuation, DVE does projection evacuation /
combines / most bias-multiplies, Pool (GPSIMD) takes 2 of 8
bias-multiplies per pair. Differential combine per-q scalars:
  x1 = (1+alpha)/S1 * U1V1 - alpha*lam/S2 * U2V1,  x2 = U2V2/S2.
"""
import sys
sys.path.insert(0, "/opt/trn_rl_repo")
import numpy as np

DIM = 512
H = 8
HD = 64
NQC = 512
NKV = 1024
MAX_DIST = 128
LAMBDA_INIT = 0.8
N_CORES = 8
SCALE = HD ** -0.5
POOL_MULT_MS = (1, 4, 7)   # k-tiles whose bias-multiply runs on GPSIMD

_COMPILED = {}


def _pv_matmuls(nc, pvs, u, ve2, hp, m, first, last):
    for qt in range(4):
        q0 = 128 * qt
        pv = pvs[qt]
        st = first and (qt % 2 == 0)
        sp = last and (qt % 2 == 1)
        nc.tensor.matmul(pv[:, 0:65], lhsT=u[:, 0, q0:q0 + 128],
                         rhs=ve2[:, hp, m, 64:129], start=st, stop=False)
        nc.tensor.matmul(pv[:, 65:194], lhsT=u[:, 1, q0:q0 + 128],
                         rhs=ve2[:, hp, m, :], start=False, stop=sp)


def _build_body(nc, tc, mybir, make_identity, tensors):
    f32 = mybir.dt.float32
    bf16 = mybir.dt.bfloat16
    AF = mybir.ActivationFunctionType
    OP = mybir.AluOpType

    xq_T = tensors["xq_T"]
    xkv_T = tensors["xkv_T"]
    wq, wk, wv, wp = (tensors[k] for k in ("wq", "wk", "wv", "wp"))
    bv = tensors["bv"]
    biasT = tensors["biasT"]
    out_T = tensors["out_T"]

    with (
        tc.tile_pool(name="const", bufs=1) as cpool,
        tc.tile_pool(name="work", bufs=1) as wpool,
        tc.tile_pool(name="stream", bufs=6) as spool,
        tc.tile_pool(name="bias", bufs=10) as bpool,
    ):
        # One merged DMA for the small constants, then inputs in
        # critical-path order: xq+wq feed the first projection, the kh=0
        # half of xkv + wk feed the first score matmuls, wv feeds V inside
        # pair 0's loop; wp is not needed until the very end.
        sm_t = cpool.tile([128, 20], f32, tag="smalls")
        nc.sync.dma_start(out=sm_t[:], in_=tensors["smalls"][:])
        bq_t = sm_t[:, 0:4]
        bk_t = sm_t[:, 4:8]
        bp_t = sm_t[:, 8:12]
        al_t = sm_t[:, 12:16]
        lam_t = sm_t[:, 16:20]
        bv_t = cpool.tile([128, DIM], f32, tag="bv")

        wq_t = cpool.tile([128, 4, DIM], bf16, tag="wq")
        wk_t = cpool.tile([128, 4, DIM], bf16, tag="wk")
        wv_t = cpool.tile([128, 4, DIM], bf16, tag="wv")
        wp_t = cpool.tile([128, 4, DIM], bf16, tag="wp")
        xq_t = wpool.tile([128, 4, NQC], bf16, tag="xq")
        xkv_t = wpool.tile([128, 4, NKV], bf16, tag="xkv")
        xq_r = xq_T[:].rearrange("(c p) n -> p c n", p=128)
        xkv_r = xkv_T[:].rearrange("(c p) n -> p c n", p=128)
        wq_r = wq[:].rearrange("(c p) o -> p c o", p=128)
        wk_r = wk[:].rearrange("(c p) o -> p c o", p=128)
        for c in range(4):
            nc.sync.dma_start(out=xq_t[:, c, :], in_=xq_r[:, c, :])
            nc.sync.dma_start(out=wq_t[:, c, :], in_=wq_r[:, c, :])
        for c in range(4):
            nc.sync.dma_start(out=xkv_t[:, c, 0:512], in_=xkv_r[:, c, 0:512])
            nc.sync.dma_start(out=wk_t[:, c, :], in_=wk_r[:, c, :])
        for c in range(4):
            nc.sync.dma_start(out=xkv_t[:, c, 512:1024],
                              in_=xkv_r[:, c, 512:1024])
        for c in range(4):
            nc.sync.dma_start(
                out=wv_t[:, c, :],
                in_=wv[:].rearrange("(c p) o -> p c o", p=128)[:, c, :])
        nc.sync.dma_start(out=bv_t[:], in_=bv[:])

        al1_t = cpool.tile([128, 4], f32, tag="al1")
        nc.vector.tensor_scalar(out=al1_t[:], in0=al_t, scalar1=1.0,
                                scalar2=None, op0=OP.add)
        alam_t = cpool.tile([128, 4, 4], f32, tag="alam")
        for hp in range(4):
            nc.vector.tensor_scalar(out=alam_t[:, hp, :], in0=al_t,
                                    scalar1=lam_t[:, hp:hp + 1], scalar2=None,
                                    op0=OP.mult)
        ident = cpool.tile([128, 128], bf16, tag="ident")
        make_identity(nc, ident[:])
        # Preload the exp spline tables during the initial DMA window.
        warm = cpool.tile([128, 2], f32, tag="warm")
        nc.vector.memset(warm[:, 0:1], 0.0)
        nc.scalar.activation(warm[:, 1:2], warm[:, 0:1], AF.Exp)

        ve2 = cpool.tile([128, 4, 8, 129], bf16, tag="ve2")
        q_sb = cpool.tile([128, 4, NQC], bf16, tag="qsb")
        k_sb = cpool.tile([128, 4, NKV], bf16, tag="ksb")
        xcat = wpool.tile([128, 4, DIM], bf16, tag="xcat")
        xcat_T = wpool.tile([128, 4, NQC], bf16, tag="xcatT")
        nc.vector.memset(ve2[:, :, :, 128:129], 1.0)
        bv_r = bv_t[:].rearrange("p (a b) -> p a b", a=4)

        with (
            tc.tile_pool(name="psum", bufs=2, space="PSUM") as ppool,
            tc.tile_pool(name="psc", bufs=2, space="PSUM") as scpool,
            tc.tile_pool(name="psacc", bufs=2, space="PSUM") as papool,
        ):
            def proj_q(hp):
                ps = ppool.tile([128, NQC], f32, tag="proj")
                for c in range(4):
                    nc.tensor.matmul(ps[:], lhsT=wq_t[:, c, 128 * hp:128 * (hp + 1)],
                                     rhs=xq_t[:, c, :], start=(c == 0), stop=(c == 3))
                nc.vector.tensor_scalar(out=q_sb[:, hp, :], in0=ps[:],
                                        scalar1=bq_t[:, hp:hp + 1], scalar2=None,
                                        op0=OP.add)

            def proj_k(hp, kh):
                ps = ppool.tile([128, NQC], f32, tag="proj")
                for c in range(4):
                    nc.tensor.matmul(
                        ps[:], lhsT=wk_t[:, c, 128 * hp:128 * (hp + 1)],
                        rhs=xkv_t[:, c, 512 * kh:512 * (kh + 1)],
                        start=(c == 0), stop=(c == 3))
                nc.vector.tensor_scalar(out=k_sb[:, hp, 512 * kh:512 * (kh + 1)],
                                        in0=ps[:], scalar1=bk_t[:, hp:hp + 1],
                                        scalar2=None, op0=OP.add)

            proj_q(0)
            proj_k(0, 0)
            for hp in range(4):
                pvbanks = []
                for _qb in range(2):
                    pv_bank = papool.tile([128, 388], f32, tag="pvacc")
                    pvbanks.append(pv_bank)
                pvs = [pvbanks[qt // 2][:, 194 * (qt % 2):194 * (qt % 2 + 1)]
                       for qt in range(4)]
                prev = None
                for m in range(8):
                    btp = bpool.tile([128, 2, NQC], bf16, tag="biasin")
                    nc.sync.dma_start(
                        out=btp[:],
                        in_=biasT[hp, m, :, :, :].rearrange("t p n -> p t n"))
                    # Row-packed score matmuls: the two heads of the pair use
                    # disjoint 64-row PE strips and adjacent PSUM banks.
                    ss = scpool.tile([128, 2, NQC], f32, tag="scores")
                    nc.tensor.matmul(
                        ss[:, 0, :],
                        lhsT=k_sb[0:64, hp, 128 * m:128 * (m + 1)],
                        rhs=q_sb[0:64, hp, :], start=True, stop=True)
                    nc.tensor.matmul(
                        ss[:, 1, :],
                        lhsT=k_sb[64:128, hp, 128 * m:128 * (m + 1)],
                        rhs=q_sb[64:128, hp, :], start=True, stop=True)
                    # Pipelined PE filler work: the rest of this pair's /
                    # the next pair's projections and pair hp-1's transposes
                    # run while ACT/DVE chew on exp and multiplies.
                    if hp == 0 and m == 0:
                        proj_k(0, 1)
                    if hp > 0 and m == 2:
                        _transpose_pair(nc, ppool, xcat, xcat_T, ident, hp - 1,
                                        f32, bf16, AF)
                    if hp < 3 and m == 5:
                        proj_q(hp + 1)
                    if hp < 3 and m == 6:
                        proj_k(hp + 1, 0)
                        proj_k(hp + 1, 1)
                    if hp == 0:
                        # V projection for k-tile m, written straight into
                        # ve2 pair blocks ([v_h2 | v_h1] column order from
                        # the host-side wv permutation).
                        vps = ppool.tile([128, DIM], f32, tag="proj")
                        for c in range(4):
                            nc.tensor.matmul(
                                vps[:], lhsT=xkv_t[:, c, 128 * m:128 * (m + 1)],
                                rhs=wv_t[:, c, :], start=(c == 0), stop=(c == 3))
                        nc.vector.tensor_tensor(
                            out=ve2[:, :, m, 0:128],
                            in0=vps[:].rearrange("p (a b) -> p a b", a=4),
                            in1=bv_r, op=OP.add)
                    uq = spool.tile([128, 2, NQC], bf16, tag="uq")
                    nc.scalar.activation(uq[:], ss[:], AF.Exp)
                    u = spool.tile([128, 2, NQC], bf16, tag="u")
                    eng = nc.gpsimd if m in POOL_MULT_MS else nc.vector
                    eng.tensor_tensor(out=u[:], in0=uq[:], in1=btp[:], op=OP.mult)
                    if prev is not None:
                        _pv_matmuls(nc, pvs, prev[0], ve2, hp, prev[1],
                                    first=(prev[1] == 0), last=False)
                    prev = (u, m)
                _pv_matmuls(nc, pvs, prev[0], ve2, hp, prev[1],
                            first=False, last=True)

                for qt in range(4):
                    pv = pvs[qt]
                    rs1 = spool.tile([128, 1], f32, tag="rs1")
                    rs2 = spool.tile([128, 1], f32, tag="rs2")
                    nc.vector.reciprocal(rs1[:], pv[:, 64:65])
                    nc.vector.reciprocal(rs2[:], pv[:, 193:194])
                    tmp1 = spool.tile([128, 64], f32, tag="tmp1")
                    nc.vector.tensor_scalar(out=tmp1[:], in0=pv[:, 0:64],
                                            scalar1=rs1[:],
                                            scalar2=al1_t[:, qt:qt + 1],
                                            op0=OP.mult, op1=OP.mult)
                    tmp2 = spool.tile([128, 64], f32, tag="tmp2")
                    nc.vector.tensor_scalar(out=tmp2[:], in0=pv[:, 129:193],
                                            scalar1=rs2[:],
                                            scalar2=alam_t[:, hp, qt:qt + 1],
                                            op0=OP.mult, op1=OP.mult)
                    nc.vector.tensor_tensor(out=xcat[:, qt, 128 * hp:128 * hp + 64],
                                            in0=tmp1[:], in1=tmp2[:],
                                            op=OP.subtract)
                    nc.scalar.activation(xcat[:, qt, 128 * hp + 64:128 * (hp + 1)],
                                         pv[:, 65:129], AF.Copy, scale=rs2[:])
            _transpose_pair(nc, ppool, xcat, xcat_T, ident, 3, f32, bf16, AF)

        for c in range(4):
            nc.sync.dma_start(
                out=wp_t[:, c, :],
                in_=wp[:].rearrange("(c p) o -> p c o", p=128)[:, c, :])
        with tc.tile_pool(name="psout", bufs=2, space="PSUM") as opool:
            for t in range(4):
                ps = opool.tile([128, NQC], f32, tag="oproj")
                for c in range(4):
                    nc.tensor.matmul(ps[:], lhsT=wp_t[:, c, 128 * t:128 * (t + 1)],
                                     rhs=xcat_T[:, c, :], start=(c == 0), stop=(c == 3))
                ot = spool.tile([128, NQC], bf16, tag="otile")
                nc.vector.tensor_scalar(out=ot[:], in0=ps[:],
                                        scalar1=bp_t[:, t:t + 1], scalar2=None,
                                        op0=OP.add)
                nc.sync.dma_start(
                    out=out_T[:].rearrange("(c p) n -> p c n", p=128)[:, t, :],
                    in_=ot[:])


def _transpose_pair(nc, ppool, xcat, xcat_T, ident, dit, f32, bf16, AF):
    """Transpose xcat[:, :, 128*dit:128*(dit+1)] into xcat_T[:, dit, :] via
    four PE transposes packed into one PSUM bank (bf16 view of a proj tile),
    then a single ACT copy."""
    pp = ppool.tile([128, NQC], f32, tag="proj")
    ppv = pp[:].bitcast(bf16)
    for qt in range(4):
        nc.tensor.transpose(out=ppv[:, 128 * qt:128 * (qt + 1)],
                            in_=xcat[:, qt, 128 * dit:128 * (dit + 1)],
                            identity=ident[:])
    nc.scalar.activation(xcat_T[:, dit, :], ppv[:, 0:NQC], AF.Copy)


def _build(reps=1):
    import concourse.bacc as bacc
    import concourse.mybir as mybir
    from concourse.tile import TileContext
    from concourse.masks import make_identity

    f32 = mybir.dt.float32
    bf16 = mybir.dt.bfloat16
    nc = bacc.Bacc("TRN2", target_bir_lowering=False, debug=False,
                   num_devices=N_CORES)
    tensors = {}
    for name, shape, dt in (
        ("xq_T", [DIM, NQC], bf16),
        ("xkv_T", [DIM, NKV], bf16),
        ("wq", [DIM, DIM], bf16),
        ("wk", [DIM, DIM], bf16),
        ("wv", [DIM, DIM], bf16),
        ("wp", [DIM, DIM], bf16),
        ("smalls", [128, 20], f32),
        ("bv", [128, DIM], f32),
        ("biasT", [4, 8, 2, 128, NQC], bf16),
    ):
        tensors[name] = nc.dram_tensor(name, shape, dt, kind="ExternalInput")
    tensors["out_T"] = nc.dram_tensor("out_T", [DIM, NQC], bf16,
                                      kind="ExternalOutput")

    with TileContext(nc) as tc:
        for _rep in range(reps):
            _build_body(nc, tc, mybir, make_identity, tensors)
    nc.compile()
    return nc


def _get_kernel(reps=1):
    key = f"k{reps}"
    if key not in _COMPILED:
        _COMPILED[key] = _build(reps)
    return _COMPILED[key]


def _to_bf16(a):
    import jax.numpy as jnp
    return np.asarray(jnp.asarray(np.asarray(a, dtype=np.float32),
                                  dtype=jnp.bfloat16))


# Head pair-interleave: projection output dim order becomes, per pair hp,
# [64 dims of head hp, 64 dims of head hp+4] for Q/K (and the xcat/Wp input
# side), and [head hp+4, head hp] for V (ve2 block order).
_PERM = np.concatenate(
    [np.r_[64 * hp:64 * (hp + 1), 64 * (hp + 4):64 * (hp + 5)] for hp in range(4)])
_PERM_V = np.concatenate(
    [np.r_[64 * (hp + 4):64 * (hp + 5), 64 * hp:64 * (hp + 1)] for hp in range(4)])


def _prep_inputs(x_q, x_kv, coords_q, coords_k, alpha_map,
                 Wq, bq, Wk, bk, Wv, bv,
                 lambda_q1, lambda_k1, lambda_q2, lambda_k2,
                 rpe_table, Wp, bp):
    x_q = np.asarray(x_q, dtype=np.float32)
    x_kv = np.asarray(x_kv, dtype=np.float32)
    coords_q = np.asarray(coords_q)
    coords_k = np.asarray(coords_k)
    alpha_map = np.asarray(alpha_map, dtype=np.float32)
    rpe = np.asarray(rpe_table, dtype=np.float32)

    lam1 = np.exp(np.sum(np.asarray(lambda_q1) * np.asarray(lambda_k1), axis=-1))
    lam2 = np.exp(np.sum(np.asarray(lambda_q2) * np.asarray(lambda_k2), axis=-1))
    lam = (lam1 - lam2 + LAMBDA_INIT).astype(np.float32)
    lam_rep = np.ascontiguousarray(np.tile(lam[None, :], (128, 1)))

    p = _PERM
    wq_l = _to_bf16((np.asarray(Wq, dtype=np.float32).T * SCALE)[:, p])
    wk_l = _to_bf16(np.asarray(Wk, dtype=np.float32).T[:, p])
    wv_l = _to_bf16(np.asarray(Wv, dtype=np.float32).T[:, _PERM_V])
    wp_l = _to_bf16(np.asarray(Wp, dtype=np.float32).T[p, :])
    bq_l = (np.asarray(bq, dtype=np.float32) * SCALE)[p].reshape(4, 128).T
    bk_l = np.asarray(bk, dtype=np.float32)[p].reshape(4, 128).T
    bv_l = np.ascontiguousarray(
        np.tile(np.asarray(bv, dtype=np.float32)[None, _PERM_V], (128, 1)))
    bp_l = np.asarray(bp, dtype=np.float32).reshape(4, 128).T
    al_l = alpha_map[:, :, 0]
    smalls = {}
    for c in range(N_CORES):
        b, qh = divmod(c, 2)
        qsl = slice(qh * NQC, (qh + 1) * NQC)
        al_c = al_l[b, qsl].reshape(4, 128).T
        smalls[c] = np.ascontiguousarray(np.concatenate(
            [bq_l, bk_l, bp_l, al_c, lam_rep[:, :4]], axis=1).astype(np.float32))

    in_maps = []
    for c in range(N_CORES):
        b, qh = divmod(c, 2)
        qsl = slice(qh * NQC, (qh + 1) * NQC)
        cq = coords_q[b, qsl]
        ck = coords_k[b]
        rel = cq[:, None, :] - ck[None, :, :] + MAX_DIST
        rel = np.clip(rel, 0, 2 * MAX_DIST)
        idx = rel[..., 0] * (2 * MAX_DIST + 1) + rel[..., 1]
        bias = np.exp(rpe[idx])                                # [512q, 1024k, 8]
        biasT = bias.transpose(2, 1, 0)                        # [8, 1024k, 512q]
        biasT = np.ascontiguousarray(
            biasT.reshape(2, 4, 8, 128, NQC).transpose(1, 2, 0, 3, 4))  # [hp, m, 2, 128, q]
        in_maps.append({
            "xq_T": _to_bf16(x_q[b, qsl].T),
            "xkv_T": _to_bf16(x_kv[b].T),
            "wq": wq_l, "wk": wk_l, "wv": wv_l, "wp": wp_l,
            "smalls": smalls[c], "bv": bv_l,
            "biasT": _to_bf16(biasT),
        })
    return in_maps


def kernel(x_q, x_kv, coords_q, coords_k, alpha_map,
           Wq, bq, Wk, bk, Wv, bv,
           lambda_q1, lambda_k1, lambda_q2, lambda_k2,
           rpe_table, Wp, bp):
    from concourse.bass_utils import run_bass_kernel_spmd

    nc = _get_kernel()
    in_maps = _prep_inputs(x_q, x_kv, coords_q, coords_k, alpha_map,
                           Wq, bq, Wk, bk, Wv, bv,
                           lambda_q1, lambda_k1, lambda_q2, lambda_k2,
                           rpe_table, Wp, bp)
    res = run_bass_kernel_spmd(nc, in_maps, list(range(N_CORES)))
    B = np.asarray(x_q).shape[0]
    out = np.zeros((B, 2 * NQC, DIM), dtype=np.float32)
    for c in range(N_CORES):
        b, qh = divmod(c, 2)
        out[b, qh * NQC:(qh + 1) * NQC] = (
            res.results[c]["out_T"].astype(np.float32).T)
    return out
